# revision 14
# baseline (speedup 1.0000x reference)
"""AttentionAugmentation2D kernel for 8 Trainium2 NeuronCores — v3.

Data-parallel over batch (B=8 -> 1 batch element per core).

Per head (H=W=32, L=1024, dh=32):
  logits[(x,y),(x',y')] = q.k + q.krw[y'-y+31] + q.krh[x'-x+31]
folded into one K=96 matmul via the augmented tensors
  QaugT = [qT*scale; skew_w(q@krwT); skew_h(q@krhT)]  (96 x 8192 bf16)
  KaugT = [kT; onehot32(y'); onehot32(x')]            (96 x 8192 bf16)
logitsT = KaugT.T @ QaugT per (head, key-tile); exp(logitsT) (bf16) is the
rhs of attention@V; the denominator rides as a ones-column of Vaug.

Engine plan: PE is the wall (~66us: QK 27 + AV 27 + rel + transposes).
ACT runs 6 of 8 exp tiles per head; DVE runs 2 via the Schraudolph bf16
bit trick (one tensor_scalar) plus the division tail; Pool evacuates
psum, scatters staging rows and builds V. Setup for heads 4-7 is
deferred into the first head iterations so the critical path to head
0's first matmul stays ~10us. finish_head PE transposes are deferred
two heads so PE never waits on the psum evacuation.
"""

import math

import numpy as np

import concourse.bass as bass
import concourse.mybir as mybir
import concourse.tile as tile
from concourse import bacc
from concourse.bass_utils import run_bass_kernel_spmd

FP = mybir.dt.float32
FPR = mybir.dt.float32r
BF = mybir.dt.bfloat16
U16 = mybir.dt.uint16
AF = mybir.ActivationFunctionType
ALU = mybir.AluOpType

B = 8
H = W = 32
NH = 8
DH = 32
L = H * W
SCALE = float(DH) ** -0.5
NT = L // 128

# exp engine per kt tile: 'A' ACT exact exp, 'D' DVE schraudolph.
EXP_PAT = ["A"] * 8
C1B = 128.0 / math.log(2.0)
C2S = 16256.5 - 9.0 - 128.0   # sample A (pre-halved); B = A+64 bits
SCH_BETA = 0.708


def _build_onehot():
    import ml_dtypes

    oh = np.zeros((64, L), dtype=np.float32)
    kk = np.arange(L)
    oh[kk % 32, kk] = 1.0
    oh[32 + kk // 32, kk] = 1.0
    return np.ascontiguousarray(np.tile(oh, (1, NH)).astype(ml_dtypes.bfloat16))


def _in_maps(inputs, key_rel_w, key_rel_h):
    import ml_dtypes

    x = np.ascontiguousarray(inputs.astype(np.float32).reshape(B, L, 3 * NH * DH))
    krwT = np.ascontiguousarray(key_rel_w.astype(np.float32).T.astype(ml_dtypes.bfloat16))
    krhT = np.ascontiguousarray(key_rel_h.astype(np.float32).T.astype(ml_dtypes.bfloat16))
    return [
        {
            "x": x[b],
            "krwT": krwT,
            "krhT": krhT,
            "oneh": _build_onehot(),
            "eye": np.eye(128, dtype=np.float32),
        }
        for b in range(B)
    ]


def _build_nc():
    nc = bacc.Bacc(
        "TRN2",
        target_bir_lowering=False,
        debug=False,
        enable_asserts=True,
        num_devices=B,
    )
    x = nc.declare_dram_parameter("x", [L, 3 * NH * DH], FP, isOutput=False)
    krw = nc.declare_dram_parameter("krwT", [DH, 2 * W - 1], BF, isOutput=False)
    krh = nc.declare_dram_parameter("krhT", [DH, 2 * H - 1], BF, isOutput=False)
    oneh = nc.declare_dram_parameter("oneh", [64, NH * L], BF, isOutput=False)
    eye = nc.declare_dram_parameter("eye", [128, 128], FP, isOutput=False)
    out = nc.declare_dram_parameter("out", [L, NH * DH], FP, isOutput=True)

    with tile.TileContext(nc) as tc:
        with (
            tc.tile_pool(name="const", bufs=1) as cp,
            tc.tile_pool(name="stg", bufs=1) as sp,
        ):
            ident = cp.tile([128, 128], FP)
            krw_sb = cp.tile([DH, 2 * W - 1], BF)
            krh_sb = cp.tile([DH, 2 * H - 1], BF)
            QaugT = cp.tile([96, NH * L], BF)
            KaugT = cp.tile([96, NH * L], BF)
            Vaug = cp.tile([128, NT, NH, DH + 1], BF)
            warm = cp.tile([DH, 1], FP)

            # --- DMAs.  ACT ring: tiny constants only.  SP ring: streaming
            # loads ordered q(h0-3), k(h0-3), onehot, q/k(h4-7), v.
            nc.sync.dma_start(out=ident, in_=eye[:])
            nc.scalar.dma_start(out=krw_sb, in_=krw[:])
            nc.scalar.dma_start(out=krh_sb, in_=krh[:])
            nc.scalar.activation(warm, krw_sb[:, 0:1], AF.Exp)  # act table load

            xr = x.rearrange("(t p) c -> p t c", p=128)
            chunks = [
                sp.tile([128, NT, 128], FP, tag=f"in{j}", name=f"chunk{j}")
                for j in range(6)
            ]
            quarters = {
                (0, 0): sp.tile([128, 4, 128], FP, tag="in0a", name="chunk0a"),
                (0, 1): sp.tile([128, 4, 128], FP, tag="in0b", name="chunk0b"),
                (2, 0): sp.tile([128, 4, 128], FP, tag="in2a", name="chunk2a"),
                (2, 1): sp.tile([128, 4, 128], FP, tag="in2b", name="chunk2b"),
            }
            nc.sync.dma_start(out=quarters[0, 0], in_=xr[:, 0:4, 0:128])
            nc.sync.dma_start(out=quarters[0, 1], in_=xr[:, 4:8, 0:128])
            nc.sync.dma_start(out=quarters[2, 0], in_=xr[:, 0:4, 256:384])
            nc.sync.dma_start(out=quarters[2, 1], in_=xr[:, 4:8, 256:384])
            nc.sync.dma_start(out=KaugT[32:96, 0:L], in_=oneh[:, 0:L])
            nc.sync.dma_start(out=KaugT[32:96, L:4*L], in_=oneh[:, L:4*L])
            nc.sync.dma_start(out=chunks[1], in_=xr[:, :, 128:256])
            nc.sync.dma_start(out=KaugT[32:96, 4*L:], in_=oneh[:, 4*L:])
            nc.sync.dma_start(out=chunks[3], in_=xr[:, :, 384:512])
            nc.sync.dma_start(out=chunks[4], in_=xr[:, :, 512:640])
            nc.sync.dma_start(out=chunks[5], in_=xr[:, :, 640:768])

            ones_col = Vaug[:, :, :, DH : DH + 1].rearrange("p t h o -> p (t h o)")
            nc.vector.memset(ones_col, 1.0)

            qstg = [
                sp.tile([128, NT, 128], BF, tag=f"qs{j}", name=f"qstg{j}")
                for j in range(2)
            ]
            kstg = [
                sp.tile([128, NT, 128], BF, tag=f"ks{j}", name=f"kstg{j}")
                for j in range(2)
            ]

            def ev_mul(eng, dst, src, s):
                if eng is nc.scalar:
                    eng.mul(dst, src, s)
                else:
                    eng.tensor_scalar_mul(dst, src, s)

            def ev_copy(eng, dst, src):
                if eng is nc.scalar:
                    eng.copy(dst, src)
                else:
                    eng.tensor_copy(dst, src)

            with (
                tc.tile_pool(name="wt", bufs=2) as wtp,
                tc.tile_pool(name="at", bufs=2) as atp,
                tc.tile_pool(name="ot", bufs=2) as otp,
                tc.tile_pool(name="sm", bufs=8) as smp,
                tc.tile_pool(name="sch", bufs=4) as schp,
                tc.tile_pool(name="ps_lt", bufs=2, space="PSUM") as ps_lt,
                tc.tile_pool(name="ps_av", bufs=2, space="PSUM") as ps_av,
                tc.tile_pool(name="ps_ms", bufs=2, space="PSUM") as ps_ms,
            ):
                # --- transposes (PE) + full-lane staging evacs
                def transpose_one(j, t, eng):
                    tp = ps_ms.tile([128, 128], FP, tag="ms")
                    if j in (0, 2):
                        src_ap = quarters[j, t // 4][:, t % 4, :]
                    else:
                        src_ap = chunks[j][:, t, :]
                    nc.tensor.transpose(tp, src_ap, ident)
                    if j < 2:
                        ev_mul(eng, qstg[j][:, t, :], tp, SCALE)
                    else:
                        ev_copy(eng, kstg[j - 2][:, t, :], tp)

                def transpose_chunk(j, engines):
                    for t in range(NT):
                        transpose_one(j, t, engines[t % len(engines)])

                def scatter(side, j, hb, eng):
                    h = 4 * j + hb
                    if side == "q":
                        src = qstg[j][32 * hb : 32 * hb + 32, :, :]
                        dst = QaugT[0:32, h * L : (h + 1) * L]
                    else:
                        src = kstg[j][32 * hb : 32 * hb + 32, :, :]
                        dst = KaugT[0:32, h * L : (h + 1) * L]
                    ev_copy(eng, dst, src.rearrange("p t c -> p (t c)"))

                transpose_chunk(0, [nc.vector])
                transpose_chunk(2, [nc.vector])
                # q scatters on DVE (4x-mode sbuf copies), k on Pool
                scatter("q", 0, 0, nc.vector)
                scatter("k", 0, 0, nc.gpsimd)
                scatter("q", 0, 1, nc.vector)
                scatter("k", 0, 1, nc.gpsimd)
                scatter("q", 0, 2, nc.vector)
                scatter("k", 0, 2, nc.gpsimd)
                scatter("q", 0, 3, nc.vector)
                scatter("k", 0, 3, nc.gpsimd)

                # --- relative logits, head-sliced.
                # QaugT[32+y',(h,x,y)] = sum_d q[d,(h,x,y)]*krw[d,y'-y+31]
                qrows_w = QaugT[0:32, :].rearrange("p (h x y2) -> p h x y2", x=H, y2=W)
                qw_dst = QaugT[32:64, :].rearrange("p (h x y2) -> p h x y2", x=H, y2=W)
                qrows_h = QaugT[0:32, :].rearrange("p (h x2 y) -> p h x2 y", x2=H, y=W)
                qh_dst = QaugT[64:96, :].rearrange("p (h x2 y) -> p h x2 y", x2=H, y=W)

                def rel_part(hlo, g0, g1, engines, hw=4):
                    hs = slice(hlo, hlo + hw)
                    ei = 0
                    for g in range(g0, g1):
                        rp = ps_ms.tile([32, 4, hw * H], FP, tag="ms")
                        for i in range(4):
                            y = 4 * g + i
                            nc.tensor.matmul(
                                rp[:, i, :],
                                lhsT=krw_sb[:, 31 - y : 63 - y],
                                rhs=qrows_w[:, hs, :, y],
                                start=True,
                                stop=True,
                            )
                        ev = rp.rearrange("p i (h x) -> p i h x", h=hw)
                        dst = qw_dst[:, hs, :, 4 * g : 4 * g + 4].rearrange(
                            "p h x i -> p i h x"
                        )
                        ev_copy(engines[ei % len(engines)], dst, ev)
                        ei += 1
                        rp = ps_ms.tile([32, 4, hw * W], FP, tag="ms")
                        for i in range(4):
                            xx = 4 * g + i
                            nc.tensor.matmul(
                                rp[:, i, :],
                                lhsT=krh_sb[:, 31 - xx : 63 - xx],
                                rhs=qrows_h[:, hs, xx, :],
                                start=True,
                                stop=True,
                            )
                        ev = rp.rearrange("p i (h y) -> p i h y", h=hw)
                        dst = qh_dst[:, hs, 4 * g : 4 * g + 4, :].rearrange(
                            "p h i y -> p i h y"
                        )
                        ev_copy(engines[ei % len(engines)], dst, ev)
                        ei += 1

                rel_part(0, 0, 8, [nc.vector])

                # V for heads 0-3 (needed by the first attention@V)
                nc.gpsimd.tensor_copy(
                    Vaug[:, :, 0:4, 0:DH],
                    chunks[4].rearrange("p t (h d) -> p t h d", d=DH),
                )

                # --- deferred setup for heads 4-7, drained 3 units/iter
                deferred = [
                    lambda: transpose_chunk(1, [nc.vector]),
                    lambda: scatter("q", 1, 0, nc.vector),
                    lambda: scatter("q", 1, 1, nc.vector),
                    lambda: transpose_chunk(3, [nc.vector]),
                    lambda: rel_part(4, 0, 4, [nc.vector], hw=2),
                    lambda: rel_part(4, 4, 8, [nc.vector], hw=2),
                    lambda: scatter("k", 1, 0, nc.gpsimd),
                    lambda: scatter("k", 1, 1, nc.gpsimd),
                    lambda: scatter("q", 1, 2, nc.vector),
                    lambda: scatter("q", 1, 3, nc.vector),
                    lambda: nc.gpsimd.tensor_copy(
                        Vaug[:, :, 4:8, 0:DH],
                        chunks[5].rearrange("p t (h d) -> p t h d", d=DH),
                    ),
                    lambda: rel_part(6, 0, 4, [nc.vector], hw=2),
                    lambda: rel_part(6, 4, 8, [nc.vector], hw=2),
                    lambda: scatter("k", 1, 2, nc.gpsimd),
                    lambda: scatter("k", 1, 3, nc.gpsimd),
                ]
                DRAIN = {0: 3, 1: 3, 2: 3, 3: 2, 4: 2, 5: 2}

                wts = {}
                avs = {}
                fin = {}

                def finish_head_stage1(h):
                    # psum -> sbuf evacuation of attn@V (Pool)
                    del wts[h]
                    av0, av1 = avs.pop(h)
                    at_sb = atp.tile([DH + 1, L], FP, tag="at")
                    nc.vector.tensor_copy(at_sb[:, 0:512], av0)
                    nc.vector.tensor_copy(at_sb[:, 512:1024], av1)
                    ot = otp.tile([128, NT, DH], FP, tag="ot", name=f"out{h}")
                    fin[h] = (at_sb, ot)

                def finish_head_ft(h, ts, mul_eng=None):
                    at_sb, ot = fin[h]
                    for t in ts:
                        ft = ps_ms.tile([128, DH + 1], FP, tag="ms")
                        nc.tensor.transpose(
                            ft,
                            at_sb[:, t * 128 : (t + 1) * 128],
                            ident[0 : DH + 1, 0 : DH + 1],
                        )
                        rcp = smp.tile([128, 1], FP, tag="rcp")
                        nc.vector.reciprocal(rcp, ft[:, DH : DH + 1])
                        nc.vector.tensor_scalar_mul(ot[:, t, :], ft[:, 0:DH], rcp)

                def finish_head_store(h):
                    _, ot = fin.pop(h)
                    out_r = out.rearrange("(t p) c -> p t c", p=128)
                    nc.sync.dma_start(
                        out=out_r[:, :, h * DH : (h + 1) * DH], in_=ot
                    )

                FT_SCHED = {2: [0, 1], 3: [2, 3], 4: [4, 5], 5: [6, 7]}

                for h in range(NH + 1):
                    if h < NH:
                        c0 = h * L
                        wts[h] = wtp.tile([128, NT * L], BF, tag="wt", name=f"wt{h}")
                    if 1 <= h <= NH:
                        avs[h - 1] = (
                            ps_av.tile([DH + 1, 512], FP, tag="av", name=f"av{h - 1}a"),
                            ps_av.tile([DH + 1, 512], FP, tag="av", name=f"av{h - 1}b"),
                        )
                    for _ in range(DRAIN.get(h, 0)):
                        if deferred:
                            deferred.pop(0)()
                    for kt in range(NT):
                        if h < NH:
                            lt = ps_lt.tile([128, L], FP, tag="lt")
                            for qc in range(2):
                                nc.tensor.matmul(
                                    lt[:, qc * 512 : (qc + 1) * 512],
                                    lhsT=KaugT[:, c0 + kt * 128 : c0 + (kt + 1) * 128],
                                    rhs=QaugT[:, c0 + qc * 512 : c0 + (qc + 1) * 512],
                                    start=True,
                                    stop=True,
                                )
                            nc.scalar.activation(
                                wts[h][:, kt * L : (kt + 1) * L], lt, AF.Exp
                            )
                        if h - 2 in fin and kt in FT_SCHED:
                            finish_head_ft(h - 2, FT_SCHED[kt])
                        if 1 <= h <= NH:
                            for qc in range(2):
                                nc.tensor.matmul(
                                    avs[h - 1][qc],
                                    lhsT=Vaug[:, kt, h - 1, :],
                                    rhs=wts[h - 1][
                                        :, kt * L + qc * 512 : kt * L + (qc + 1) * 512
                                    ],
                                    start=(kt == 0),
                                    stop=(kt == NT - 1),
                                )
                    if h >= 1:
                        finish_head_stage1(h - 1)
                    if h == NH:
                        finish_head_ft(h - 1, range(NT))
                        finish_head_store(h - 1)
                    if h - 2 in fin:
                        finish_head_store(h - 2)
    nc.compile()
    return nc


_NC_CACHE = None


def kernel(inputs: np.ndarray, key_rel_w: np.ndarray, key_rel_h: np.ndarray) -> np.ndarray:
    global _NC_CACHE
    if _NC_CACHE is None:
        _NC_CACHE = _build_nc()
    nc = _NC_CACHE
    in_maps = _in_maps(inputs, key_rel_w, key_rel_h)
    res = run_bass_kernel_spmd(nc, in_maps, list(range(B)))
    out = np.stack([res.results[b]["out"] for b in range(B)], axis=0)
    return np.ascontiguousarray(out.reshape(B, H, W, NH * DH).astype(np.float32))
